# revision 22
# baseline (speedup 1.0000x reference)
"""Morphological dilation (max-plus 3x3 depthwise conv) on 8 Trainium2 cores.

out[b,c,y,x] = max_{i,j in 3x3} ( x_pad[b,c,y+i,x+j] + se[c,i,j] ),
x: [16,64,256,256] f32, se: [64,3,3] f32, pad=1 with CVAL=-10000.

Sharding: pure data parallel. Core k takes batches {2k, 2k+1}; the 2*64
(batch,channel) pairs map onto the 128 SBUF partitions, so se[c,i,j] is a
per-partition scalar. Spatial dims live on the free axis.

Measured DVE modes (fp16, 0.96 GHz): scalar_tensor_tensor is 1x only;
tensor_scalar is 4x when 4B-aligned (2x at odd offsets); tensor_tensor is 2x.
ACT (1.2 GHz) does Identity(in + per-partition bias) at 1x and is otherwise
idle. So each tap is add+max with the adds split between engines:
  - 3 taps (j=0, 4B-aligned): DVE tensor_scalar add (4x) + tensor_tensor max (2x)
  - 6 taps (j=1 odd, j=2): ACT Identity+bias add into ping-pong tmp tiles,
    DVE tensor_tensor max (2x)
DVE ~41us/block vs ACT ~43us/block -> balanced pipeline.

Sync-wait budgets are 1 per instruction for every compute/DMA encoding used
here, so cross-engine handoffs go through 1-element "gate" ops that carry the
single foreign-semaphore wait (the consumer then only needs its own-engine
wait): DVE memset gates before each TT that reads an ACT tmp, ACT 1-element
Identity gates for tmp-slot reuse and input-chunk waits. x is fully
SBUF-resident (one persistent tile, 5 chunked loads serially chained on one
HWDGE queue); each block's store is split into two half-height HWDGE DMAs on
separate queues (queues recycle every ~3.5 blocks) so the acc WAR frees
~2.7us sooner. A post-pass splits any remaining multi-wait instruction (the
framework epilogue drain) into single-wait drains.

Third-engine options explored and RULED OUT (2026-08-09 session):
  - Pool (nc.gpsimd) ALU compute: walrus codegen rejects TensorScalarPtr /
    TensorTensor on Pool for core_v3 ("Instruction engine check failed") —
    Pool only runs memset/iota/copy/DMA + prebuilt Q7 ucode (topk etc).
    The `standard` GPSIMD library DOES ship tensor_tensor ucode, but only
    via the Bacc (target_bir_lowering) pipeline, which can't be driven
    through plain run_bass_kernel_spmd (defers reg alloc -> walrus fails).
  - DMA compute (cce_op): HW supports Max for all dtypes
    (has_valid_dma_cce_inout_dtype_nc_v1), but the BIR verifier only
    accepts ADD (max/min/mult all rejected in every mode x dst combo).
  - PE: can produce shifted+bias candidates via shifted-identity matmul +
    ones-row bias, but outputs land fp32 in PSUM; casting back costs one
    ACT/DVE pass per candidate = the add it saved. No max on PE/PSUM.
So DVE+ACT carry all 17 ops/elem; this kernel sits at ~95% of that
two-engine roofline. Measured serial-chain rates (ns/elem/partition, fp16,
[128,8192] tiles): DVE TT max .67, DVE TS add .42, DVE STT 1.20, ACT 1.00,
SWDGE SBUF->SBUF copy 1.20. Removing the DVE gate memsets (waits land on
the big compute ops instead) costs +6us — gates are load-bearing.
Tuning sweep: sp=4 (spdiv=10) best 394.1us; sp=6: 396.7; sp=8: 404.2;
sp=0: 408.5; nblocks 12: 400.1; nblocks 10: SBUF overflow; first chunk 16
rows: 403.8 (28 best).

Trace findings (neuron-profile, core 0, 392.7us run): DVE is 100% busy with
zero >300ns gaps after +85us — the kernel is at the DVE roofline in steady
state. All recoverable slack is ~25us of DVE gaps in the first ~85us
(preamble ~7us, first-chunk DMA ~10us incl queue overheads, ACT table load,
and ACT lag cascading into early fold stalls) plus a ~6us framework
barrier/drain tail. Attempts that all landed within the 392.7-395.3us noise
band or worse: precise f23 slot-gate (ACT starts a block earlier), se load
on its own queue, leaner JIT load chunks [26,30,50x4], smaller first block
12 + tail 4 (399.2 — per-block overhead dominates), NOGATES (401.1 — waits
must stay on 1-elem gates, not compute ops). Parallelizing loads across
queues is pointless: the serial chain already runs at full DMA bandwidth
(~305GB/s).

Round 3 (391.5us best): remaining DVE gaps are chunk-0 start latency
(~16us: 7us framework preamble + DMA issue + 5.7us transfer), an ACT-lag
cascade over blocks 0-3 (ACT's first add can't start before chunk 0), and
a ~7us end barrier. Small-head block layouts ([16,24x10]: 399.2) lose —
the default's small TAIL is what matters. Split half-height stores (this
version) measured equal to single stores (391.8 vs 391.5) and drop the
SWDGE path.

Round 4 (2026-08-11, 377.8us): THE BIG FIND — a hardware slow-mode trip.
Microbenches (mb3/mb7/mb8): an in-place DVE op (dst==src0, e.g. the acc
folds) overlapping a SATURATED DMA load stream latches the core into a
~1.2x-slower mode FOR THE REST OF THE RUN, even after the DMA drains
(TT@16384: 8692 -> 10429ns, held through later clean phases). Out-of-place
TT under the same stream, or the same in-place ops under ~40%-duty spread
loads, do NOT trip. All per-op rates measured solo match theory exactly
(TT fp16 2x: (FD/2+58)/0.96; TS 4x: (FD/4+58)/0.96; ACT: (FD+352)/1.2;
data content, engine concurrency, striding, stores: all no effect). The
old 391-475us runs were ALL tripped (TT 0.65-0.67 ns/elem); the "measured
rates" above are trip-mode rates. The trip is
PROBABILISTIC (one ungated-56-row config tripped on run 1; another ran
15x clean) and PERSISTS ACROSS NEFF EXECUTIONS (the next run after a
tripped one measured 452us before the state cleared). Fix: only a tiny
ungated load prefix ([6,24] rows) that lands BEFORE the first fold;
every later chunk gated on an f01 >= blk0 with ~2 blocks of lead
(1-block leads reintroduce 3-6us chunk-wait stalls). Untripped
in-situ rates: TT@6144 3354ns (0.546), TS strided 1814 (0.295), ACT 5399
(0.879). Post-fix the two engines re-balance at ~32.3us/block each
(Vector 92.8% / Scalar 89.8% union-busy) — this IS the two-engine floor:
8 TT folds (26.8us/block) cannot move off DVE, and ACT's 6 adds just
cover the rest. DMA-cce add offload nets +0.4us (DVE-bound either way).
Tiny 4-row head block + 6-row first chunk starts DVE at ~9us (-2.3us).
Baked config (median ~372us over 10 runs, band 371.2-374.1; device
run-to-run variance ~+-1.5us): blocks [4,24x10,12], loads
[6,24,40,62,62,62] with chunk0 on q0 and chunk1 on q2 IN PARALLEL (both
land before the first fold; -4.2us vs serial), chunk2 kept small (40
rows) so it beats blk2's need at the ~170GB/s effective rate of strided
row-loads (-2us), sp=4, tail sp=6, tail store split 4-way (neutral).
Sweeps at untripped rates, all worse: sp=2 378.9, sp=0 378.6, sp=6
382.0, NOGATES 387.4, no-WAR-gates 373.3 (gates stay load-bearing),
nblocks=10 SBUF-overflows by 0.4KB, 1-block load leads 395.2, 3 store
queues neutral, se-load emitted after chunks 380.3 (program-order moves
perturb Tile's whole schedule), F45 374.1, first chunk 5 rows neutral.
Fixed remains: ~7.5us framework preamble, ~8us sem-teardown epilogue,
~6us warmup, ~4us end drain; Vector is gap-free from 29us to 365us —
this is the two-engine floor. NOT attempted (negative EV near grading):
DMA-accumulate add offload (dst=dst+src SBUF->SBUF RMW) — would cut
~1.8us/block of DVE TS but needs +12KB SBUF (none left), ~65% DMA duty,
and SBUF-writing DMA over in-place folds = the trip recipe.
"""

import os
import numpy as np

B, C, H, W = 16, 64, 256, 256
NCORES = 8
P = 128  # partitions = (B // NCORES) * C
CVAL = -10000.0
KH = KW = 3

_DTYPE = os.environ.get("DILATION_DTYPE", "f16")

_nc_cache = {}
LAST_RESULTS = None  # BassKernelResults of the most recent run (for profiling)

# instruction name -> forced HWDGE queue index (consulted by the patched
# TileClockTick._assign_tick during scheduling)
_FORCED_HW_QUEUE = {}
_ASSIGN_PATCHED = False

# taps: (i, j) with per-tap scalar index t = 3*i + j
_DVE_TAPS = [(0, 0), (1, 0), (2, 0)]  # j=0 -> 4B-aligned reads
_ACT_TAPS = [(0, 1), (1, 1), (2, 1), (0, 2), (1, 2), (2, 2)]


def _patch_queue_assignment():
    global _ASSIGN_PATCHED
    if _ASSIGN_PATCHED:
        return
    import concourse.tile_sem_assignment as tsa

    orig = tsa.TileClockTick._assign_tick

    def _assign_tick(self, inst):
        forced = _FORCED_HW_QUEUE.get(getattr(inst, "name", None))
        if forced is None:
            return orig(self, inst)
        save = self.next_hw_dma_idx
        self.next_hw_dma_idx = forced
        try:
            return orig(self, inst)
        finally:
            self.next_hw_dma_idx = save

    tsa.TileClockTick._assign_tick = _assign_tick
    _ASSIGN_PATCHED = True


def _split_excess_waits(nc, mybir, max_waits: int = 1):
    """Walrus's per-encoding sync-wait slots are scarce (1 for most ops used
    here). Hoist all but `max_waits` waits of any instruction onto freshly
    inserted same-engine Drain instructions placed right before it."""
    n = 0
    for bb in nc.main_func.blocks:
        insts = bb.instructions
        i = 0
        while i < len(insts):
            ins = insts[i]
            si = ins.sync_info
            if si is not None and len(si.on_wait) > max_waits:
                waits = list(si.on_wait)
                keep = waits[-max_waits:]
                spill = waits[:-max_waits]
                new_insts = []
                for w in spill:
                    d = mybir.InstDrain(name=f"wsplit-{n}", ins=[], outs=[])
                    n += 1
                    d.engine = ins.engine
                    d.sync_info = mybir.SyncInfo(on_wait=[w], on_update=[])
                    new_insts.append(d)
                ins.sync_info = mybir.SyncInfo(
                    on_wait=keep, on_update=list(si.on_update)
                )
                insts[i:i] = new_insts
                i += len(new_insts)
            i += 1
        bb.instructions = insts


def _block_sizes(h: int, nblocks: int):
    base = h // nblocks
    rem = h - base * nblocks
    return [base + (1 if i < rem else 0) for i in range(nblocks)]


def _build(dtype_tag: str, h: int = H, nblocks: int = 11, nloads: int = 5, nslots: int = 3, split_waits: bool = True):
    import concourse.bass as bass
    import concourse.mybir as mybir
    from concourse.tile import TileContext, add_dep_helper

    _patch_queue_assignment()
    _FORCED_HW_QUEUE.clear()

    assert dtype_tag == "f16", "v4 layout is fp16-only"
    dt = mybir.dt.float16
    f32 = mybir.dt.float32
    add = mybir.AluOpType.add
    vmax = mybir.AluOpType.max
    ident = mybir.ActivationFunctionType.Identity

    _STQN = int(os.environ.get("DILATION_STQN", "7"))
    nc = bass.Bass(
        trn_type="TRN2",
        num_swdge_queues=int(os.environ.get("DILATION_SWDGEQ", "1")),
    )
    x_d = nc.declare_dram_parameter("x", [P, h, W], dt, isOutput=False)
    se_d = nc.declare_dram_parameter("sep", [P, KH * KW], f32, isOutput=False)
    out_d = nc.declare_dram_parameter("out", [P, h, W], dt, isOutput=True)

    # all block sizes EVEN: the DVE 4x tensor_scalar mode needs even dims
    # (odd-row blocks measurably fall back to 2x). Tiny 4-row head block:
    # DVE starts ~9us after launch on a 6-row first chunk instead of ~18us
    # on a 28-row one (378.1us vs 380.1us).
    base = 2 * ((h + 2 * nblocks - 1) // (2 * nblocks))
    blocks = [4] + [base] * (nblocks - 1)
    blocks.append(h - sum(blocks))  # small even tail (also drains fast)
    if os.environ.get("DILATION_BLOCKS"):
        blocks = [int(v) for v in os.environ["DILATION_BLOCKS"].split(",")]
    assert all(b % 2 == 0 and b > 0 for b in blocks) and sum(blocks) == h
    first = int(os.environ.get("DILATION_FIRST", "6"))
    # tiny ungated prefix (chunk0 on q0, chunk1 on q2 in parallel) lands
    # before the first fold; chunk2 kept small so it beats blk2's need even
    # at the ~170GB/s effective rate of these strided row-loads
    loads = [first, 24, 40] + _block_sizes(h - first - 64, 3)
    if os.environ.get("DILATION_LOADS"):
        loads = [int(v) for v in os.environ["DILATION_LOADS"].split(",")]
    assert sum(loads) == h

    with TileContext(nc) as tc:
        with (
            tc.tile_pool(name="const", bufs=1) as cpool,
            tc.tile_pool(name="xp", bufs=1) as xpool,
            tc.tile_pool(name="accp", bufs=2) as apool,
            tc.tile_pool(name="tmpp", bufs=1) as tpool,
        ):
            se_t = cpool.tile([P, KH * KW], f32, name="se_t")
            se_dma = nc.sync.dma_start(out=se_t[:], in_=se_d[:])
            # own queue: don't chain the first x chunk behind the se load
            _FORCED_HW_QUEUE[se_dma.ins.name] = 5

            # One persistent padded-x tile: xt row t = padded-input row t.
            xt = xpool.tile([P, h + 2, W + 2], dt, name="xt")
            nc.vector.memset(xt[:, :, 0:1], CVAL)
            nc.vector.memset(xt[:, :, W + 1 : W + 2], CVAL)
            nc.vector.memset(xt[:, 0:1, :], CVAL)
            nc.vector.memset(xt[:, h + 1 : h + 2, :], CVAL)

            # Chunked loads, serially chained on HWDGE queue 0.
            load_dmas = []
            load_top = []  # last loaded input row (exclusive) per chunk
            y0 = 0
            for rows in loads:
                ld = nc.sync.dma_start(
                    out=xt[:, y0 + 1 : y0 + rows + 1, 1 : W + 1],
                    in_=x_d[:, y0 : y0 + rows, :],
                )
                # chunk1 rides its own queue, parallel with chunk0: both
                # land before the first fold (the ungated prefix stays tiny)
                _FORCED_HW_QUEUE[ld.ins.name] = 2 if len(load_dmas) == 1 else 0
                load_dmas.append(ld)
                y0 += rows
                load_top.append(y0)
            # Trip-and-hold avoidance (measured, mb3/mb7/mb8): an in-place DVE
            # op overlapping a SATURATED load stream latches the whole rest of
            # the run into a ~1.2x-slower mode (TT 8692->10429ns for the
            # remainder, even after DMA drains). Spread loads (~40% duty) or
            # out-of-place folds both avoid the trip. Gate chunk c behind an
            # early block's compute so the load stream never saturates while
            # folds run: release chunk c at block 2c-3 (consumed at block
            # 2c-1, ~2 blocks of lead).
            load_release = {}
            if not os.environ.get("DILATION_NOLDGATE"):
                lead = int(os.environ.get("DILATION_LDLEAD", "2"))
                bt = 0
                first_need = {}
                for bi, br in enumerate(blocks):
                    nt = min(bt + br + 1, h)
                    ci = next(i for i, top in enumerate(load_top) if top >= nt)
                    first_need.setdefault(ci, bi)
                    bt += br
                # chunks 0-1 stay ungated but are sized to land BEFORE the
                # first in-place fold; every later chunk is gated (release at
                # blk0's f01 at the earliest) so the load stream never
                # saturates while folds run. The trip is probabilistic: one
                # ungated-56-row config tripped on its first run while 15
                # runs of another didn't — keep the overlap at ~zero.
                for c in range(2, len(load_dmas)):
                    rb = max(first_need.get(c, 0) - lead, 0)
                    load_release.setdefault(rb, []).append(c)

            # scratch tiles for gates (tiny 1-element targets)
            dve_scr = cpool.tile([P, 3 * len(blocks)], dt, name="dve_scr")
            act_scr = cpool.tile([P, 4], dt, name="act_scr")
            act_src = cpool.tile([P, 1], dt, name="act_src")
            nc.vector.memset(act_src[:], 0.0)

            # ping-pong tmp tiles for the ACT adds
            maxrows = max(blocks)
            tmps = [
                tpool.tile([P, maxrows, W], dt, name=f"tmp{i}") for i in range(nslots)
            ]
            tmp_reader = [None] * nslots  # [0] = last fold of prev block

            out_dmas = []
            y0 = 0
            tmp_idx = 0
            for blk, rows in enumerate(blocks):
                # deepest load chunk this block needs (bottom halo row is
                # input row y0+rows; the queue-0 chain covers earlier chunks)
                need_top = min(y0 + rows + 1, h)
                ldi = next(i for i, top in enumerate(load_top) if top >= need_top)

                acc = apool.tile([P, rows, W], dt, name="acc")
                # DVE-side gates: gw absorbs the store whose acc slot this
                # block reuses, gx the input-chunk wait.
                if not os.environ.get("DILATION_NOGATES"):
                    gx = nc.vector.memset(dve_scr[:, 3 * blk : 3 * blk + 1], 0.0)
                    add_dep_helper(gx.ins, load_dmas[ldi].ins, reason="input chunk")
                # Hoist the first tmp_d tap's TS add (no acc dependency)
                # ahead of the store-WAR gates: hides ~1.6us of the acc-WAR
                # wait on blocks still inside the early-phase cascade (trace:
                # 7.3us DVE stall at blk 3 on a store-half sem).
                tmp_d = tpool.tile([P, rows, W], dt, name="tmp_d")
                (h_i, h_j) = _DVE_TAPS[1]
                nc.vector.tensor_scalar(
                    tmp_d[:],
                    xt[:, y0 + h_i : y0 + h_i + rows, h_j : h_j + W],
                    se_t[:, 3 * h_i + h_j : 3 * h_i + h_j + 1],
                    None,
                    add,
                )
                if blk >= 2 and not os.environ.get("DILATION_NOWARGATES"):
                    gw = nc.vector.memset(dve_scr[:, 3 * blk + 1 : 3 * blk + 2], 0.0)
                    add_dep_helper(gw.ins, out_dmas[blk - 2][0].ins, reason="acc WAR")
                    gw2 = nc.vector.memset(dve_scr[:, 3 * blk + 2 : 3 * blk + 3], 0.0)
                    add_dep_helper(gw2.ins, out_dmas[blk - 2][1].ins, reason="acc WAR2")
                # ACT-side gate for the input chunk
                ga = nc.scalar.activation(
                    act_scr[:, 0:1], act_src[:, 0:1], ident, bias=se_t[:, 0:1]
                )
                add_dep_helper(ga.ins, load_dmas[ldi].ins, reason="input chunk/ACT")

                def act_add(tap, gate_tt, split=0):
                    """Emit one ACT add into the next tmp slot; gate_tt (if
                    set) is a fold whose completion frees the slot (and, by
                    the cumulative DVE semaphore, every earlier fold). With
                    split>0 and an even-offset tap, the bottom `split` rows
                    are computed by an otherwise-idle DVE 4x tensor_scalar
                    instead (rebalances the ACT-bound pipeline); the fold's
                    DVE-chain wait covers that part for free."""
                    nonlocal tmp_idx
                    t_i, t_j = tap
                    ti = tmp_idx % nslots
                    tmp_idx += 1
                    sidx = 3 * t_i + t_j
                    if gate_tt is not None:
                        gt = nc.scalar.activation(
                            act_scr[:, 1:2], act_src[:, 0:1], ident,
                            bias=se_t[:, 0:1],
                        )
                        add_dep_helper(gt.ins, gate_tt.ins, reason="tmp WAR gate")
                    arows = rows - split
                    a = nc.scalar.activation(
                        tmps[ti][:, 0:arows, :],
                        xt[:, y0 + t_i : y0 + t_i + arows, t_j : t_j + W],
                        ident,
                        bias=se_t[:, sidx : sidx + 1],
                    )
                    if split:
                        nc.vector.tensor_scalar(
                            tmps[ti][:, arows:rows, :],
                            xt[:, y0 + t_i + arows : y0 + t_i + rows, t_j : t_j + W],
                            se_t[:, sidx : sidx + 1],
                            None,
                            add,
                        )
                    return ti, a

                def act_fold_pair(p0, p1):
                    """Fold two finished ACT tmps; one gate on the later add
                    covers both (same-engine retirement is in-order)."""
                    if not os.environ.get("DILATION_NOGATES"):
                        gm = nc.vector.memset(dve_scr[:, 3 * blk : 3 * blk + 1], 0.0)
                        add_dep_helper(gm.ins, p1[1].ins, reason="ACT pair ready")
                    nc.vector.tensor_tensor(
                        acc[:], acc[:], tmps[p0[0]][:, 0:rows, :], vmax
                    )
                    return nc.vector.tensor_tensor(
                        acc[:], acc[:], tmps[p1[0]][:, 0:rows, :], vmax
                    )

                # DVE-only taps: aligned TS init + 2x (TS add -> tmp_d, TT max)
                # (first tmp_d TS was hoisted above the store-WAR gates)
                (i0, j0) = _DVE_TAPS[0]
                nc.vector.tensor_scalar(
                    acc[:],
                    xt[:, y0 + i0 : y0 + i0 + rows, j0 : j0 + W],
                    se_t[:, 3 * i0 + j0 : 3 * i0 + j0 + 1],
                    None,
                    add,
                )
                nc.vector.tensor_tensor(acc[:], acc[:], tmp_d[:], vmax)
                (t_i, t_j) = _DVE_TAPS[2]
                sidx = 3 * t_i + t_j
                nc.vector.tensor_scalar(
                    tmp_d[:],
                    xt[:, y0 + t_i : y0 + t_i + rows, t_j : t_j + W],
                    se_t[:, sidx : sidx + 1],
                    None,
                    add,
                )
                nc.vector.tensor_tensor(acc[:], acc[:], tmp_d[:], vmax)
                # paired folds with slot refill: adds a0(s0) a1(s1) fold01,
                # a2(s2) a3(s0) fold23, a4(s1) a5(s2) fold45. Slot-reuse gates
                # wait on the later fold of the freeing pair (cumulative).
                p0 = act_add(_ACT_TAPS[0], tmp_reader[0])
                p1 = act_add(_ACT_TAPS[1], None)
                f01 = act_fold_pair(p0, p1)
                for c in load_release.get(blk, []):
                    add_dep_helper(
                        load_dmas[c].ins, f01.ins, reason="load throttle"
                    )
                p2 = act_add(_ACT_TAPS[2], None)
                # sp=4 (spdiv=10) measured best: DVE is the binding engine, so
                # keep the ACT->DVE row-split small (395us vs 397 at sp=6,
                # 404 at sp=8, 408 at sp=0).
                spdiv = int(os.environ.get("DILATION_SPDIV", "10"))
                sp = 2 * (rows // spdiv)  # even bottom slice for the DVE part
                if blk == len(blocks) - 1:
                    # last block: DVE idles at the drain; absorb more ACT rows
                    tsp = int(os.environ.get("DILATION_TAILSP", "6"))
                    sp = min(rows, 2 * (tsp // 2))
                if blk == 0 and os.environ.get("DILATION_SP0"):
                    # Tested: DVE-heavy first block (sp0=16) = 399.6us, +8us
                    # WORSE — DVE is the slower engine, so absorbing ACT rows
                    # in block 0 delays its own folds 1:1; the early ACT-wait
                    # gaps are already mostly pipeline-hidden.
                    sp = int(os.environ["DILATION_SP0"])
                p3 = act_add(_ACT_TAPS[3], f01, split=sp)  # s0 freed by f01
                f23 = act_fold_pair(p2, p3)
                p4 = act_add(_ACT_TAPS[4], None)  # s1 freed by f01 (observed)
                p5 = act_add(_ACT_TAPS[5], f23, split=sp)  # s2 freed by f23
                f45 = act_fold_pair(p4, p5)
                # s0's last reader is f23's 2nd TT (f45 only reads s1/s2), so
                # gating next block's a0 on f23 lets ACT start a block earlier
                # relative to DVE; a1/a2 WAR vs f45 ride their auto region
                # deps. (Trace showed ACT-lag-induced DVE fold stalls in the
                # first ~85us.)
                tmp_reader[0] = f45 if os.environ.get("DILATION_F45") else f23

                # two half-height stores on separate queues: acc's WAR
                # frees ~2.7us sooner, damping the early-phase store-WAR
                # cascade; all stores HWDGE (queues recycle every ~3.5 blocks)
                if blk == len(blocks) - 1 and rows % 4 == 0:
                    # tail: 4 parallel quarter-stores shorten the final drain
                    q4 = rows // 4
                    tail_stores = []
                    for qi in range(4):
                        od = nc.sync.dma_start(
                            out=out_d[:, y0 + qi * q4 : y0 + (qi + 1) * q4, :],
                            in_=acc[:, qi * q4 : (qi + 1) * q4, :],
                        )
                        _FORCED_HW_QUEUE[od.ins.name] = 1 + (2 * blk + qi) % _STQN
                        tail_stores.append(od)
                    out_dmas.append((tail_stores[0], tail_stores[-1]))
                else:
                    h1 = rows // 2
                    od1 = nc.sync.dma_start(
                        out=out_d[:, y0 : y0 + h1, :], in_=acc[:, 0:h1, :]
                    )
                    _FORCED_HW_QUEUE[od1.ins.name] = 1 + (2 * blk) % _STQN
                    od2 = nc.sync.dma_start(
                        out=out_d[:, y0 + h1 : y0 + rows, :], in_=acc[:, h1:rows, :]
                    )
                    _FORCED_HW_QUEUE[od2.ins.name] = 1 + (2 * blk + 1) % _STQN
                    out_dmas.append((od1, od2))
                y0 += rows

    if split_waits:
        _split_excess_waits(nc, mybir)
    return nc


def _get_nc():
    key = (_DTYPE,)
    if key not in _nc_cache:
        _nc_cache[key] = _build(
            _DTYPE, nblocks=int(os.environ.get("DILATION_NBLOCKS", "11"))
        )
    return _nc_cache[key]


def kernel(x: np.ndarray, se: np.ndarray) -> np.ndarray:
    global LAST_RESULTS
    from concourse.bass_utils import run_bass_kernel_spmd

    np_dt = np.float16 if _DTYPE == "f16" else np.float32
    x = np.asarray(x)
    se = np.asarray(se)
    xs = np.ascontiguousarray(x).reshape(NCORES, P, H, W).astype(np_dt)
    sep = np.ascontiguousarray(
        np.tile(np.asarray(se, np.float32).reshape(C, KH * KW), (P // C, 1))
    )

    nc = _get_nc()
    in_maps = [{"x": xs[k], "sep": sep} for k in range(NCORES)]
    trace = bool(os.environ.get("DILATION_TRACE"))
    kwargs = {}
    if trace:
        kwargs["trace"] = True
        tmpdir = os.environ.get("DILATION_TRACE_DIR")
        if tmpdir:
            kwargs["tmpdir"] = tmpdir
    res = run_bass_kernel_spmd(nc, in_maps, list(range(NCORES)), **kwargs)
    LAST_RESULTS = res
    out = np.stack([res.results[k]["out"] for k in range(NCORES)])
    return out.reshape(B, C, H, W).astype(np.float32)



# revision 23
# speedup vs baseline: 1.0011x; 1.0011x over previous
"""Morphological dilation (max-plus 3x3 depthwise conv) on 8 Trainium2 cores.

out[b,c,y,x] = max_{i,j in 3x3} ( x_pad[b,c,y+i,x+j] + se[c,i,j] ),
x: [16,64,256,256] f32, se: [64,3,3] f32, pad=1 with CVAL=-10000.

Sharding: pure data parallel. Core k takes batches {2k, 2k+1}; the 2*64
(batch,channel) pairs map onto the 128 SBUF partitions, so se[c,i,j] is a
per-partition scalar. Spatial dims live on the free axis.

Measured DVE modes (fp16, 0.96 GHz): scalar_tensor_tensor is 1x only;
tensor_scalar is 4x when 4B-aligned (2x at odd offsets); tensor_tensor is 2x.
ACT (1.2 GHz) does Identity(in + per-partition bias) at 1x and is otherwise
idle. So each tap is add+max with the adds split between engines:
  - 3 taps (j=0, 4B-aligned): DVE tensor_scalar add (4x) + tensor_tensor max (2x)
  - 6 taps (j=1 odd, j=2): ACT Identity+bias add into ping-pong tmp tiles,
    DVE tensor_tensor max (2x)
DVE ~41us/block vs ACT ~43us/block -> balanced pipeline.

Sync-wait budgets are 1 per instruction for every compute/DMA encoding used
here, so cross-engine handoffs go through 1-element "gate" ops that carry the
single foreign-semaphore wait (the consumer then only needs its own-engine
wait): DVE memset gates before each TT that reads an ACT tmp, ACT 1-element
Identity gates for tmp-slot reuse and input-chunk waits. x is fully
SBUF-resident (one persistent tile, 5 chunked loads serially chained on one
HWDGE queue); each block's store is split into two half-height HWDGE DMAs on
separate queues (queues recycle every ~3.5 blocks) so the acc WAR frees
~2.7us sooner. A post-pass splits any remaining multi-wait instruction (the
framework epilogue drain) into single-wait drains.

Third-engine options explored and RULED OUT (2026-08-09 session):
  - Pool (nc.gpsimd) ALU compute: walrus codegen rejects TensorScalarPtr /
    TensorTensor on Pool for core_v3 ("Instruction engine check failed") —
    Pool only runs memset/iota/copy/DMA + prebuilt Q7 ucode (topk etc).
    The `standard` GPSIMD library DOES ship tensor_tensor ucode, but only
    via the Bacc (target_bir_lowering) pipeline, which can't be driven
    through plain run_bass_kernel_spmd (defers reg alloc -> walrus fails).
  - DMA compute (cce_op): HW supports Max for all dtypes
    (has_valid_dma_cce_inout_dtype_nc_v1), but the BIR verifier only
    accepts ADD (max/min/mult all rejected in every mode x dst combo).
  - PE: can produce shifted+bias candidates via shifted-identity matmul +
    ones-row bias, but outputs land fp32 in PSUM; casting back costs one
    ACT/DVE pass per candidate = the add it saved. No max on PE/PSUM.
So DVE+ACT carry all 17 ops/elem; this kernel sits at ~95% of that
two-engine roofline. Measured serial-chain rates (ns/elem/partition, fp16,
[128,8192] tiles): DVE TT max .67, DVE TS add .42, DVE STT 1.20, ACT 1.00,
SWDGE SBUF->SBUF copy 1.20. Removing the DVE gate memsets (waits land on
the big compute ops instead) costs +6us — gates are load-bearing.
Tuning sweep: sp=4 (spdiv=10) best 394.1us; sp=6: 396.7; sp=8: 404.2;
sp=0: 408.5; nblocks 12: 400.1; nblocks 10: SBUF overflow; first chunk 16
rows: 403.8 (28 best).

Trace findings (neuron-profile, core 0, 392.7us run): DVE is 100% busy with
zero >300ns gaps after +85us — the kernel is at the DVE roofline in steady
state. All recoverable slack is ~25us of DVE gaps in the first ~85us
(preamble ~7us, first-chunk DMA ~10us incl queue overheads, ACT table load,
and ACT lag cascading into early fold stalls) plus a ~6us framework
barrier/drain tail. Attempts that all landed within the 392.7-395.3us noise
band or worse: precise f23 slot-gate (ACT starts a block earlier), se load
on its own queue, leaner JIT load chunks [26,30,50x4], smaller first block
12 + tail 4 (399.2 — per-block overhead dominates), NOGATES (401.1 — waits
must stay on 1-elem gates, not compute ops). Parallelizing loads across
queues is pointless: the serial chain already runs at full DMA bandwidth
(~305GB/s).

Round 3 (391.5us best): remaining DVE gaps are chunk-0 start latency
(~16us: 7us framework preamble + DMA issue + 5.7us transfer), an ACT-lag
cascade over blocks 0-3 (ACT's first add can't start before chunk 0), and
a ~7us end barrier. Small-head block layouts ([16,24x10]: 399.2) lose —
the default's small TAIL is what matters. Split half-height stores (this
version) measured equal to single stores (391.8 vs 391.5) and drop the
SWDGE path.

Round 4 (2026-08-11, 377.8us): THE BIG FIND — a hardware slow-mode trip.
Microbenches (mb3/mb7/mb8): an in-place DVE op (dst==src0, e.g. the acc
folds) overlapping a SATURATED DMA load stream latches the core into a
~1.2x-slower mode FOR THE REST OF THE RUN, even after the DMA drains
(TT@16384: 8692 -> 10429ns, held through later clean phases). Out-of-place
TT under the same stream, or the same in-place ops under ~40%-duty spread
loads, do NOT trip. All per-op rates measured solo match theory exactly
(TT fp16 2x: (FD/2+58)/0.96; TS 4x: (FD/4+58)/0.96; ACT: (FD+352)/1.2;
data content, engine concurrency, striding, stores: all no effect). The
old 391-475us runs were ALL tripped (TT 0.65-0.67 ns/elem); the "measured
rates" above are trip-mode rates. The trip is
PROBABILISTIC (one ungated-56-row config tripped on run 1; another ran
15x clean) and PERSISTS ACROSS NEFF EXECUTIONS (the next run after a
tripped one measured 452us before the state cleared). Fix: only a tiny
ungated load prefix ([6,24] rows) that lands BEFORE the first fold;
every later chunk gated on an f01 >= blk0 with ~2 blocks of lead
(1-block leads reintroduce 3-6us chunk-wait stalls). Untripped
in-situ rates: TT@6144 3354ns (0.546), TS strided 1814 (0.295), ACT 5399
(0.879). Post-fix the two engines re-balance at ~32.3us/block each
(Vector 92.8% / Scalar 89.8% union-busy) — this IS the two-engine floor:
8 TT folds (26.8us/block) cannot move off DVE, and ACT's 6 adds just
cover the rest. DMA-cce add offload nets +0.4us (DVE-bound either way).
Tiny 4-row head block + 6-row first chunk starts DVE at ~9us (-2.3us).
Baked config (median ~372us over 10 runs, band 371.2-374.1; device
run-to-run variance ~+-1.5us): blocks [4,24x10,12], loads
[6,24,40,62,62,62] with chunk0 on q0 and chunk1 on q2 IN PARALLEL (both
land before the first fold; -4.2us vs serial), chunk2 kept small (40
rows) so it beats blk2's need at the ~170GB/s effective rate of strided
row-loads (-2us), sp=4, tail sp=6, tail store split 4-way (neutral).
Sweeps at untripped rates, all worse: sp=2 378.9, sp=0 378.6, sp=6
382.0, NOGATES 387.4, no-WAR-gates 373.3 (gates stay load-bearing),
nblocks=10 SBUF-overflows by 0.4KB, 1-block load leads 395.2, 3 store
queues neutral, se-load emitted after chunks 380.3 (program-order moves
perturb Tile's whole schedule), F45 374.1, first chunk 5 rows neutral.
Fixed remains: ~7.5us framework preamble, ~8us sem-teardown epilogue,
~6us warmup, ~4us end drain; Vector is gap-free from 29us to 365us —
this is the two-engine floor. NOT attempted (negative EV near grading):
DMA-accumulate add offload (dst=dst+src SBUF->SBUF RMW) — would cut
~1.8us/block of DVE TS but needs +12KB SBUF (none left), ~65% DMA duty,
and SBUF-writing DMA over in-place folds = the trip recipe.
"""

import os
import numpy as np

B, C, H, W = 16, 64, 256, 256
NCORES = 8
P = 128  # partitions = (B // NCORES) * C
CVAL = -10000.0
KH = KW = 3

_DTYPE = os.environ.get("DILATION_DTYPE", "f16")

_nc_cache = {}
LAST_RESULTS = None  # BassKernelResults of the most recent run (for profiling)

# instruction name -> forced HWDGE queue index (consulted by the patched
# TileClockTick._assign_tick during scheduling)
_FORCED_HW_QUEUE = {}
_ASSIGN_PATCHED = False

# taps: (i, j) with per-tap scalar index t = 3*i + j
_DVE_TAPS = [(0, 0), (1, 0), (2, 0)]  # j=0 -> 4B-aligned reads
_ACT_TAPS = [(0, 1), (1, 1), (2, 1), (0, 2), (1, 2), (2, 2)]


def _patch_queue_assignment():
    global _ASSIGN_PATCHED
    if _ASSIGN_PATCHED:
        return
    import concourse.tile_sem_assignment as tsa

    orig = tsa.TileClockTick._assign_tick

    def _assign_tick(self, inst):
        forced = _FORCED_HW_QUEUE.get(getattr(inst, "name", None))
        if forced is None:
            return orig(self, inst)
        save = self.next_hw_dma_idx
        self.next_hw_dma_idx = forced
        try:
            return orig(self, inst)
        finally:
            self.next_hw_dma_idx = save

    tsa.TileClockTick._assign_tick = _assign_tick
    _ASSIGN_PATCHED = True


def _split_excess_waits(nc, mybir, max_waits: int = 1):
    """Walrus's per-encoding sync-wait slots are scarce (1 for most ops used
    here). Hoist all but `max_waits` waits of any instruction onto freshly
    inserted same-engine Drain instructions placed right before it."""
    n = 0
    for bb in nc.main_func.blocks:
        insts = bb.instructions
        i = 0
        while i < len(insts):
            ins = insts[i]
            si = ins.sync_info
            if si is not None and len(si.on_wait) > max_waits:
                waits = list(si.on_wait)
                keep = waits[-max_waits:]
                spill = waits[:-max_waits]
                new_insts = []
                for w in spill:
                    d = mybir.InstDrain(name=f"wsplit-{n}", ins=[], outs=[])
                    n += 1
                    d.engine = ins.engine
                    d.sync_info = mybir.SyncInfo(on_wait=[w], on_update=[])
                    new_insts.append(d)
                ins.sync_info = mybir.SyncInfo(
                    on_wait=keep, on_update=list(si.on_update)
                )
                insts[i:i] = new_insts
                i += len(new_insts)
            i += 1
        bb.instructions = insts


def _block_sizes(h: int, nblocks: int):
    base = h // nblocks
    rem = h - base * nblocks
    return [base + (1 if i < rem else 0) for i in range(nblocks)]


def _build(dtype_tag: str, h: int = H, nblocks: int = 11, nloads: int = 5, nslots: int = 3, split_waits: bool = True):
    import concourse.bass as bass
    import concourse.mybir as mybir
    from concourse.tile import TileContext, add_dep_helper

    _patch_queue_assignment()
    _FORCED_HW_QUEUE.clear()

    assert dtype_tag == "f16", "v4 layout is fp16-only"
    dt = mybir.dt.float16
    f32 = mybir.dt.float32
    add = mybir.AluOpType.add
    vmax = mybir.AluOpType.max
    ident = mybir.ActivationFunctionType.Identity

    _STQN = int(os.environ.get("DILATION_STQN", "7"))
    nc = bass.Bass(
        trn_type="TRN2",
        num_swdge_queues=int(os.environ.get("DILATION_SWDGEQ", "1")),
    )
    x_d = nc.declare_dram_parameter("x", [P, h, W], dt, isOutput=False)
    se_d = nc.declare_dram_parameter("sep", [P, KH * KW], f32, isOutput=False)
    out_d = nc.declare_dram_parameter("out", [P, h, W], dt, isOutput=True)

    # all block sizes EVEN: the DVE 4x tensor_scalar mode needs even dims
    # (odd-row blocks measurably fall back to 2x). Tiny 4-row head block:
    # DVE starts ~9us after launch on a 6-row first chunk instead of ~18us
    # on a 28-row one (378.1us vs 380.1us).
    base = 2 * ((h + 2 * nblocks - 1) // (2 * nblocks))
    blocks = [4] + [base] * (nblocks - 1)
    blocks.append(h - sum(blocks))  # small even tail (also drains fast)
    if os.environ.get("DILATION_BLOCKS"):
        blocks = [int(v) for v in os.environ["DILATION_BLOCKS"].split(",")]
    assert all(b % 2 == 0 and b > 0 for b in blocks) and sum(blocks) == h
    first = int(os.environ.get("DILATION_FIRST", "6"))
    # tiny ungated prefix (chunk0 on q0, chunk1 on q2 in parallel) lands
    # before the first fold; chunk2 kept small so it beats blk2's need even
    # at the ~170GB/s effective rate of these strided row-loads
    loads = [first, 24, 40] + _block_sizes(h - first - 64, 3)
    if os.environ.get("DILATION_LOADS"):
        loads = [int(v) for v in os.environ["DILATION_LOADS"].split(",")]
    assert sum(loads) == h

    with TileContext(nc) as tc:
        with (
            tc.tile_pool(name="const", bufs=1) as cpool,
            tc.tile_pool(name="xp", bufs=1) as xpool,
            tc.tile_pool(name="accp", bufs=2) as apool,
            tc.tile_pool(name="tmpp", bufs=1) as tpool,
        ):
            se_t = cpool.tile([P, KH * KW], f32, name="se_t")
            se_dma = nc.sync.dma_start(out=se_t[:], in_=se_d[:])
            # own queue: don't chain the first x chunk behind the se load
            _FORCED_HW_QUEUE[se_dma.ins.name] = 5

            # One persistent padded-x tile: xt row t = padded-input row t.
            xt = xpool.tile([P, h + 2, W + 2], dt, name="xt")
            nc.vector.memset(xt[:, :, 0:1], CVAL)
            nc.vector.memset(xt[:, :, W + 1 : W + 2], CVAL)
            nc.vector.memset(xt[:, 0:1, :], CVAL)
            nc.vector.memset(xt[:, h + 1 : h + 2, :], CVAL)

            # Chunked loads, serially chained on HWDGE queue 0.
            load_dmas = []
            load_top = []  # last loaded input row (exclusive) per chunk
            y0 = 0
            for rows in loads:
                ld = nc.sync.dma_start(
                    out=xt[:, y0 + 1 : y0 + rows + 1, 1 : W + 1],
                    in_=x_d[:, y0 : y0 + rows, :],
                )
                # chunk1 rides its own queue, parallel with chunk0: both
                # land before the first fold (the ungated prefix stays tiny)
                _FORCED_HW_QUEUE[ld.ins.name] = 2 if len(load_dmas) == 1 else 0
                load_dmas.append(ld)
                y0 += rows
                load_top.append(y0)
            # Trip-and-hold avoidance (measured, mb3/mb7/mb8): an in-place DVE
            # op overlapping a SATURATED load stream latches the whole rest of
            # the run into a ~1.2x-slower mode (TT 8692->10429ns for the
            # remainder, even after DMA drains). Spread loads (~40% duty) or
            # out-of-place folds both avoid the trip. Gate chunk c behind an
            # early block's compute so the load stream never saturates while
            # folds run: release chunk c at block 2c-3 (consumed at block
            # 2c-1, ~2 blocks of lead).
            load_release = {}
            if not os.environ.get("DILATION_NOLDGATE"):
                lead = int(os.environ.get("DILATION_LDLEAD", "2"))
                bt = 0
                first_need = {}
                for bi, br in enumerate(blocks):
                    nt = min(bt + br + 1, h)
                    ci = next(i for i, top in enumerate(load_top) if top >= nt)
                    first_need.setdefault(ci, bi)
                    bt += br
                # chunks 0-1 stay ungated but are sized to land BEFORE the
                # first in-place fold; every later chunk is gated (release at
                # blk0's f01 at the earliest) so the load stream never
                # saturates while folds run. The trip is probabilistic: one
                # ungated-56-row config tripped on its first run while 15
                # runs of another didn't — keep the overlap at ~zero.
                for c in range(2, len(load_dmas)):
                    rb = max(first_need.get(c, 0) - lead, 0)
                    load_release.setdefault(rb, []).append(c)

            # scratch tiles for gates (tiny 1-element targets)
            dve_scr = cpool.tile([P, 3 * len(blocks)], dt, name="dve_scr")
            act_scr = cpool.tile([P, 4], dt, name="act_scr")
            act_src = cpool.tile([P, 1], dt, name="act_src")
            nc.vector.memset(act_src[:], 0.0)

            # ping-pong tmp tiles for the ACT adds
            maxrows = max(blocks)
            tmps = [
                tpool.tile([P, maxrows, W], dt, name=f"tmp{i}") for i in range(nslots)
            ]
            tmp_reader = [None] * nslots  # [0] = last fold of prev block

            out_dmas = []
            y0 = 0
            tmp_idx = 0
            for blk, rows in enumerate(blocks):
                # deepest load chunk this block needs (bottom halo row is
                # input row y0+rows; the queue-0 chain covers earlier chunks)
                need_top = min(y0 + rows + 1, h)
                ldi = next(i for i, top in enumerate(load_top) if top >= need_top)

                acc = apool.tile([P, rows, W], dt, name="acc")
                # DVE-side gates: gw absorbs the store whose acc slot this
                # block reuses, gx the input-chunk wait.
                if not os.environ.get("DILATION_NOGATES"):
                    gx = nc.vector.memset(dve_scr[:, 3 * blk : 3 * blk + 1], 0.0)
                    add_dep_helper(gx.ins, load_dmas[ldi].ins, reason="input chunk")
                # Hoist the first tmp_d tap's TS add (no acc dependency)
                # ahead of the store-WAR gates: hides ~1.6us of the acc-WAR
                # wait on blocks still inside the early-phase cascade (trace:
                # 7.3us DVE stall at blk 3 on a store-half sem).
                tmp_d = tpool.tile([P, rows, W], dt, name="tmp_d")
                (h_i, h_j) = _DVE_TAPS[1]
                nc.vector.tensor_scalar(
                    tmp_d[:],
                    xt[:, y0 + h_i : y0 + h_i + rows, h_j : h_j + W],
                    se_t[:, 3 * h_i + h_j : 3 * h_i + h_j + 1],
                    None,
                    add,
                )
                if blk >= 2 and not os.environ.get("DILATION_NOWARGATES"):
                    gw = nc.vector.memset(dve_scr[:, 3 * blk + 1 : 3 * blk + 2], 0.0)
                    add_dep_helper(gw.ins, out_dmas[blk - 2][0].ins, reason="acc WAR")
                    gw2 = nc.vector.memset(dve_scr[:, 3 * blk + 2 : 3 * blk + 3], 0.0)
                    add_dep_helper(gw2.ins, out_dmas[blk - 2][1].ins, reason="acc WAR2")
                # ACT-side gate for the input chunk (294ns each on the
                # bottleneck engine; NOACTGATES drops them in favor of auto
                # region deps + wait-split drains)
                if not os.environ.get("DILATION_NOACTGATES"):
                    ga = nc.scalar.activation(
                        act_scr[:, 0:1], act_src[:, 0:1], ident, bias=se_t[:, 0:1]
                    )
                    add_dep_helper(ga.ins, load_dmas[ldi].ins, reason="input chunk/ACT")

                def act_add(tap, gate_tt, split=0):
                    """Emit one ACT add into the next tmp slot; gate_tt (if
                    set) is a fold whose completion frees the slot (and, by
                    the cumulative DVE semaphore, every earlier fold). With
                    split>0 and an even-offset tap, the bottom `split` rows
                    are computed by an otherwise-idle DVE 4x tensor_scalar
                    instead (rebalances the ACT-bound pipeline); the fold's
                    DVE-chain wait covers that part for free."""
                    nonlocal tmp_idx
                    t_i, t_j = tap
                    ti = tmp_idx % nslots
                    tmp_idx += 1
                    sidx = 3 * t_i + t_j
                    if gate_tt is not None and not os.environ.get(
                        "DILATION_NOACTGATES"
                    ):
                        gt = nc.scalar.activation(
                            act_scr[:, 1:2], act_src[:, 0:1], ident,
                            bias=se_t[:, 0:1],
                        )
                        add_dep_helper(gt.ins, gate_tt.ins, reason="tmp WAR gate")
                    arows = rows - split
                    a = nc.scalar.activation(
                        tmps[ti][:, 0:arows, :],
                        xt[:, y0 + t_i : y0 + t_i + arows, t_j : t_j + W],
                        ident,
                        bias=se_t[:, sidx : sidx + 1],
                    )
                    if split:
                        nc.vector.tensor_scalar(
                            tmps[ti][:, arows:rows, :],
                            xt[:, y0 + t_i + arows : y0 + t_i + rows, t_j : t_j + W],
                            se_t[:, sidx : sidx + 1],
                            None,
                            add,
                        )
                    return ti, a

                def act_fold_pair(p0, p1):
                    """Fold two finished ACT tmps; one gate on the later add
                    covers both (same-engine retirement is in-order)."""
                    if not os.environ.get("DILATION_NOGATES"):
                        gm = nc.vector.memset(dve_scr[:, 3 * blk : 3 * blk + 1], 0.0)
                        add_dep_helper(gm.ins, p1[1].ins, reason="ACT pair ready")
                    nc.vector.tensor_tensor(
                        acc[:], acc[:], tmps[p0[0]][:, 0:rows, :], vmax
                    )
                    return nc.vector.tensor_tensor(
                        acc[:], acc[:], tmps[p1[0]][:, 0:rows, :], vmax
                    )

                # DVE-only taps: aligned TS init + 2x (TS add -> tmp_d, TT max)
                # (first tmp_d TS was hoisted above the store-WAR gates)
                (i0, j0) = _DVE_TAPS[0]
                nc.vector.tensor_scalar(
                    acc[:],
                    xt[:, y0 + i0 : y0 + i0 + rows, j0 : j0 + W],
                    se_t[:, 3 * i0 + j0 : 3 * i0 + j0 + 1],
                    None,
                    add,
                )
                nc.vector.tensor_tensor(acc[:], acc[:], tmp_d[:], vmax)
                (t_i, t_j) = _DVE_TAPS[2]
                sidx = 3 * t_i + t_j
                nc.vector.tensor_scalar(
                    tmp_d[:],
                    xt[:, y0 + t_i : y0 + t_i + rows, t_j : t_j + W],
                    se_t[:, sidx : sidx + 1],
                    None,
                    add,
                )
                nc.vector.tensor_tensor(acc[:], acc[:], tmp_d[:], vmax)
                # paired folds with slot refill: adds a0(s0) a1(s1) fold01,
                # a2(s2) a3(s0) fold23, a4(s1) a5(s2) fold45. Slot-reuse gates
                # wait on the later fold of the freeing pair (cumulative).
                p0 = act_add(_ACT_TAPS[0], tmp_reader[0])
                p1 = act_add(_ACT_TAPS[1], None)
                f01 = act_fold_pair(p0, p1)
                for c in load_release.get(blk, []):
                    add_dep_helper(
                        load_dmas[c].ins, f01.ins, reason="load throttle"
                    )
                p2 = act_add(_ACT_TAPS[2], None)
                # sp=4 (spdiv=10) measured best: DVE is the binding engine, so
                # keep the ACT->DVE row-split small (395us vs 397 at sp=6,
                # 404 at sp=8, 408 at sp=0).
                spdiv = int(os.environ.get("DILATION_SPDIV", "10"))
                sp = 2 * (rows // spdiv)  # even bottom slice for the DVE part
                if blk == len(blocks) - 1:
                    # last block: DVE idles at the drain; absorb more ACT rows
                    tsp = int(os.environ.get("DILATION_TAILSP", "6"))
                    sp = min(rows, 2 * (tsp // 2))
                if blk == 0 and os.environ.get("DILATION_SP0"):
                    # Tested: DVE-heavy first block (sp0=16) = 399.6us, +8us
                    # WORSE — DVE is the slower engine, so absorbing ACT rows
                    # in block 0 delays its own folds 1:1; the early ACT-wait
                    # gaps are already mostly pipeline-hidden.
                    sp = int(os.environ["DILATION_SP0"])
                p3 = act_add(_ACT_TAPS[3], f01, split=sp)  # s0 freed by f01
                f23 = act_fold_pair(p2, p3)
                p4 = act_add(_ACT_TAPS[4], None)  # s1 freed by f01 (observed)
                p5 = act_add(_ACT_TAPS[5], f23, split=sp)  # s2 freed by f23
                f45 = act_fold_pair(p4, p5)
                # s0's last reader is f23's 2nd TT (f45 only reads s1/s2), so
                # gating next block's a0 on f23 lets ACT start a block earlier
                # relative to DVE; a1/a2 WAR vs f45 ride their auto region
                # deps. (Trace showed ACT-lag-induced DVE fold stalls in the
                # first ~85us.)
                tmp_reader[0] = f45 if os.environ.get("DILATION_F45") else f23

                # two half-height stores on separate queues: acc's WAR
                # frees ~2.7us sooner, damping the early-phase store-WAR
                # cascade; all stores HWDGE (queues recycle every ~3.5 blocks)
                if blk == len(blocks) - 1 and rows % 4 == 0:
                    # tail: 4 parallel quarter-stores shorten the final drain
                    q4 = rows // 4
                    tail_stores = []
                    for qi in range(4):
                        od = nc.sync.dma_start(
                            out=out_d[:, y0 + qi * q4 : y0 + (qi + 1) * q4, :],
                            in_=acc[:, qi * q4 : (qi + 1) * q4, :],
                        )
                        _FORCED_HW_QUEUE[od.ins.name] = 1 + (2 * blk + qi) % _STQN
                        tail_stores.append(od)
                    out_dmas.append((tail_stores[0], tail_stores[-1]))
                else:
                    h1 = rows // 2
                    od1 = nc.sync.dma_start(
                        out=out_d[:, y0 : y0 + h1, :], in_=acc[:, 0:h1, :]
                    )
                    _FORCED_HW_QUEUE[od1.ins.name] = 1 + (2 * blk) % _STQN
                    od2 = nc.sync.dma_start(
                        out=out_d[:, y0 + h1 : y0 + rows, :], in_=acc[:, h1:rows, :]
                    )
                    _FORCED_HW_QUEUE[od2.ins.name] = 1 + (2 * blk + 1) % _STQN
                    out_dmas.append((od1, od2))
                y0 += rows

    if split_waits:
        _split_excess_waits(nc, mybir)
    return nc


def _get_nc():
    key = (_DTYPE,)
    if key not in _nc_cache:
        _nc_cache[key] = _build(
            _DTYPE, nblocks=int(os.environ.get("DILATION_NBLOCKS", "11"))
        )
    return _nc_cache[key]


def kernel(x: np.ndarray, se: np.ndarray) -> np.ndarray:
    global LAST_RESULTS
    from concourse.bass_utils import run_bass_kernel_spmd

    np_dt = np.float16 if _DTYPE == "f16" else np.float32
    x = np.asarray(x)
    se = np.asarray(se)
    xs = np.ascontiguousarray(x).reshape(NCORES, P, H, W).astype(np_dt)
    sep = np.ascontiguousarray(
        np.tile(np.asarray(se, np.float32).reshape(C, KH * KW), (P // C, 1))
    )

    nc = _get_nc()
    in_maps = [{"x": xs[k], "sep": sep} for k in range(NCORES)]
    trace = bool(os.environ.get("DILATION_TRACE"))
    kwargs = {}
    if trace:
        kwargs["trace"] = True
        tmpdir = os.environ.get("DILATION_TRACE_DIR")
        if tmpdir:
            kwargs["tmpdir"] = tmpdir
    res = run_bass_kernel_spmd(nc, in_maps, list(range(NCORES)), **kwargs)
    LAST_RESULTS = res
    out = np.stack([res.results[k]["out"] for k in range(NCORES)])
    return out.reshape(B, C, H, W).astype(np.float32)



# revision 25
# speedup vs baseline: 1.0024x; 1.0014x over previous
"""Morphological dilation (max-plus 3x3 depthwise conv) on 8 Trainium2 cores.

out[b,c,y,x] = max_{i,j in 3x3} ( x_pad[b,c,y+i,x+j] + se[c,i,j] ),
x: [16,64,256,256] f32, se: [64,3,3] f32, pad=1 with CVAL=-10000.

Sharding: pure data parallel. Core k takes batches {2k, 2k+1}; the 2*64
(batch,channel) pairs map onto the 128 SBUF partitions, so se[c,i,j] is a
per-partition scalar. Spatial dims live on the free axis.

Measured DVE modes (fp16, 0.96 GHz): scalar_tensor_tensor is 1x only;
tensor_scalar is 4x when 4B-aligned (2x at odd offsets); tensor_tensor is 2x.
ACT (1.2 GHz) does Identity(in + per-partition bias) at 1x and is otherwise
idle. So each tap is add+max with the adds split between engines:
  - 3 taps (j=0, 4B-aligned): DVE tensor_scalar add (4x) + tensor_tensor max (2x)
  - 6 taps (j=1 odd, j=2): ACT Identity+bias add into ping-pong tmp tiles,
    DVE tensor_tensor max (2x)
DVE ~41us/block vs ACT ~43us/block -> balanced pipeline.

Sync-wait budgets are 1 per instruction for every compute/DMA encoding used
here, so cross-engine handoffs go through 1-element "gate" ops that carry the
single foreign-semaphore wait (the consumer then only needs its own-engine
wait): DVE memset gates before each TT that reads an ACT tmp, ACT 1-element
Identity gates for tmp-slot reuse and input-chunk waits. x is fully
SBUF-resident (one persistent tile, 5 chunked loads serially chained on one
HWDGE queue); each block's store is split into two half-height HWDGE DMAs on
separate queues (queues recycle every ~3.5 blocks) so the acc WAR frees
~2.7us sooner. A post-pass splits any remaining multi-wait instruction (the
framework epilogue drain) into single-wait drains.

Third-engine options explored and RULED OUT (2026-08-09 session):
  - Pool (nc.gpsimd) ALU compute: walrus codegen rejects TensorScalarPtr /
    TensorTensor on Pool for core_v3 ("Instruction engine check failed") —
    Pool only runs memset/iota/copy/DMA + prebuilt Q7 ucode (topk etc).
    The `standard` GPSIMD library DOES ship tensor_tensor ucode, but only
    via the Bacc (target_bir_lowering) pipeline, which can't be driven
    through plain run_bass_kernel_spmd (defers reg alloc -> walrus fails).
  - DMA compute (cce_op): HW supports Max for all dtypes
    (has_valid_dma_cce_inout_dtype_nc_v1), but the BIR verifier only
    accepts ADD (max/min/mult all rejected in every mode x dst combo).
  - PE: can produce shifted+bias candidates via shifted-identity matmul +
    ones-row bias, but outputs land fp32 in PSUM; casting back costs one
    ACT/DVE pass per candidate = the add it saved. No max on PE/PSUM.
So DVE+ACT carry all 17 ops/elem; this kernel sits at ~95% of that
two-engine roofline. Measured serial-chain rates (ns/elem/partition, fp16,
[128,8192] tiles): DVE TT max .67, DVE TS add .42, DVE STT 1.20, ACT 1.00,
SWDGE SBUF->SBUF copy 1.20. Removing the DVE gate memsets (waits land on
the big compute ops instead) costs +6us — gates are load-bearing.
Tuning sweep: sp=4 (spdiv=10) best 394.1us; sp=6: 396.7; sp=8: 404.2;
sp=0: 408.5; nblocks 12: 400.1; nblocks 10: SBUF overflow; first chunk 16
rows: 403.8 (28 best).

Trace findings (neuron-profile, core 0, 392.7us run): DVE is 100% busy with
zero >300ns gaps after +85us — the kernel is at the DVE roofline in steady
state. All recoverable slack is ~25us of DVE gaps in the first ~85us
(preamble ~7us, first-chunk DMA ~10us incl queue overheads, ACT table load,
and ACT lag cascading into early fold stalls) plus a ~6us framework
barrier/drain tail. Attempts that all landed within the 392.7-395.3us noise
band or worse: precise f23 slot-gate (ACT starts a block earlier), se load
on its own queue, leaner JIT load chunks [26,30,50x4], smaller first block
12 + tail 4 (399.2 — per-block overhead dominates), NOGATES (401.1 — waits
must stay on 1-elem gates, not compute ops). Parallelizing loads across
queues is pointless: the serial chain already runs at full DMA bandwidth
(~305GB/s).

Round 3 (391.5us best): remaining DVE gaps are chunk-0 start latency
(~16us: 7us framework preamble + DMA issue + 5.7us transfer), an ACT-lag
cascade over blocks 0-3 (ACT's first add can't start before chunk 0), and
a ~7us end barrier. Small-head block layouts ([16,24x10]: 399.2) lose —
the default's small TAIL is what matters. Split half-height stores (this
version) measured equal to single stores (391.8 vs 391.5) and drop the
SWDGE path.

Round 4 (2026-08-11, 377.8us): THE BIG FIND — a hardware slow-mode trip.
Microbenches (mb3/mb7/mb8): an in-place DVE op (dst==src0, e.g. the acc
folds) overlapping a SATURATED DMA load stream latches the core into a
~1.2x-slower mode FOR THE REST OF THE RUN, even after the DMA drains
(TT@16384: 8692 -> 10429ns, held through later clean phases). Out-of-place
TT under the same stream, or the same in-place ops under ~40%-duty spread
loads, do NOT trip. All per-op rates measured solo match theory exactly
(TT fp16 2x: (FD/2+58)/0.96; TS 4x: (FD/4+58)/0.96; ACT: (FD+352)/1.2;
data content, engine concurrency, striding, stores: all no effect). The
old 391-475us runs were ALL tripped (TT 0.65-0.67 ns/elem); the "measured
rates" above are trip-mode rates. The trip is
PROBABILISTIC (one ungated-56-row config tripped on run 1; another ran
15x clean) and PERSISTS ACROSS NEFF EXECUTIONS (the next run after a
tripped one measured 452us before the state cleared). Fix: only a tiny
ungated load prefix ([6,24] rows) that lands BEFORE the first fold;
every later chunk gated on an f01 >= blk0 with ~2 blocks of lead
(1-block leads reintroduce 3-6us chunk-wait stalls). Untripped
in-situ rates: TT@6144 3354ns (0.546), TS strided 1814 (0.295), ACT 5399
(0.879). Post-fix the two engines re-balance at ~32.3us/block each
(Vector 92.8% / Scalar 89.8% union-busy) — this IS the two-engine floor:
8 TT folds (26.8us/block) cannot move off DVE, and ACT's 6 adds just
cover the rest. DMA-cce add offload nets +0.4us (DVE-bound either way).
Tiny 4-row head block + 6-row first chunk starts DVE at ~9us (-2.3us).
Baked config (median ~372us over 10 runs, band 371.2-374.1; device
run-to-run variance ~+-1.5us): blocks [4,24x10,12], loads
[6,24,40,62,62,62] with chunk0 on q0 and chunk1 on q2 IN PARALLEL (both
land before the first fold; -4.2us vs serial), chunk2 kept small (40
rows) so it beats blk2's need at the ~170GB/s effective rate of strided
row-loads (-2us), sp=4, tail sp=6, tail store split 4-way (neutral).
Sweeps at untripped rates, all worse: sp=2 378.9, sp=0 378.6, sp=6
382.0, NOGATES 387.4, no-WAR-gates 373.3 (gates stay load-bearing),
nblocks=10 SBUF-overflows by 0.4KB, 1-block load leads 395.2, 3 store
queues neutral, se-load emitted after chunks 380.3 (program-order moves
perturb Tile's whole schedule), F45 374.1, first chunk 5 rows neutral.
Fixed remains: ~7.5us framework preamble, ~8us sem-teardown epilogue,
~6us warmup, ~4us end drain; Vector is gap-free from 29us to 365us —
this is the two-engine floor. NOT attempted (negative EV near grading):
DMA-accumulate add offload (dst=dst+src SBUF->SBUF RMW) — would cut
~1.8us/block of DVE TS but needs +12KB SBUF (none left), ~65% DMA duty,
and SBUF-writing DMA over in-place folds = the trip recipe.
"""

import os
import numpy as np

B, C, H, W = 16, 64, 256, 256
NCORES = 8
P = 128  # partitions = (B // NCORES) * C
CVAL = -10000.0
KH = KW = 3

_DTYPE = os.environ.get("DILATION_DTYPE", "f16")

_nc_cache = {}
LAST_RESULTS = None  # BassKernelResults of the most recent run (for profiling)

# instruction name -> forced HWDGE queue index (consulted by the patched
# TileClockTick._assign_tick during scheduling)
_FORCED_HW_QUEUE = {}
_ASSIGN_PATCHED = False

# taps: (i, j) with per-tap scalar index t = 3*i + j
_DVE_TAPS = [(0, 0), (1, 0), (2, 0)]  # j=0 -> 4B-aligned reads
_ACT_TAPS = [(0, 1), (1, 1), (2, 1), (0, 2), (1, 2), (2, 2)]


def _patch_queue_assignment():
    global _ASSIGN_PATCHED
    if _ASSIGN_PATCHED:
        return
    import concourse.tile_sem_assignment as tsa

    orig = tsa.TileClockTick._assign_tick

    def _assign_tick(self, inst):
        forced = _FORCED_HW_QUEUE.get(getattr(inst, "name", None))
        if forced is None:
            return orig(self, inst)
        save = self.next_hw_dma_idx
        self.next_hw_dma_idx = forced
        try:
            return orig(self, inst)
        finally:
            self.next_hw_dma_idx = save

    tsa.TileClockTick._assign_tick = _assign_tick
    _ASSIGN_PATCHED = True


def _split_excess_waits(nc, mybir, max_waits: int = 1):
    """Walrus's per-encoding sync-wait slots are scarce (1 for most ops used
    here). Hoist all but `max_waits` waits of any instruction onto freshly
    inserted same-engine Drain instructions placed right before it."""
    n = 0
    for bb in nc.main_func.blocks:
        insts = bb.instructions
        i = 0
        while i < len(insts):
            ins = insts[i]
            si = ins.sync_info
            if si is not None and len(si.on_wait) > max_waits:
                waits = list(si.on_wait)
                keep = waits[-max_waits:]
                spill = waits[:-max_waits]
                new_insts = []
                for w in spill:
                    d = mybir.InstDrain(name=f"wsplit-{n}", ins=[], outs=[])
                    n += 1
                    d.engine = ins.engine
                    d.sync_info = mybir.SyncInfo(on_wait=[w], on_update=[])
                    new_insts.append(d)
                ins.sync_info = mybir.SyncInfo(
                    on_wait=keep, on_update=list(si.on_update)
                )
                insts[i:i] = new_insts
                i += len(new_insts)
            i += 1
        bb.instructions = insts


def _block_sizes(h: int, nblocks: int):
    base = h // nblocks
    rem = h - base * nblocks
    return [base + (1 if i < rem else 0) for i in range(nblocks)]


def _build(dtype_tag: str, h: int = H, nblocks: int = 11, nloads: int = 5, nslots: int = 3, split_waits: bool = True):
    import concourse.bass as bass
    import concourse.mybir as mybir
    from concourse.tile import TileContext, add_dep_helper

    _patch_queue_assignment()
    _FORCED_HW_QUEUE.clear()

    assert dtype_tag == "f16", "v4 layout is fp16-only"
    dt = mybir.dt.float16
    f32 = mybir.dt.float32
    add = mybir.AluOpType.add
    vmax = mybir.AluOpType.max
    ident = mybir.ActivationFunctionType.Identity

    _STQN = int(os.environ.get("DILATION_STQN", "7"))
    nc = bass.Bass(
        trn_type="TRN2",
        num_swdge_queues=int(os.environ.get("DILATION_SWDGEQ", "1")),
    )
    x_d = nc.declare_dram_parameter("x", [P, h, W], dt, isOutput=False)
    se_d = nc.declare_dram_parameter("sep", [P, KH * KW], f32, isOutput=False)
    out_d = nc.declare_dram_parameter("out", [P, h, W], dt, isOutput=True)

    # all block sizes EVEN: the DVE 4x tensor_scalar mode needs even dims
    # (odd-row blocks measurably fall back to 2x). Tiny 4-row head block:
    # DVE starts ~9us after launch on a 6-row first chunk instead of ~18us
    # on a 28-row one (378.1us vs 380.1us).
    base = 2 * ((h + 2 * nblocks - 1) // (2 * nblocks))
    blocks = [4] + [base] * (nblocks - 1)
    blocks.append(h - sum(blocks))  # small even tail (also drains fast)
    if os.environ.get("DILATION_BLOCKS"):
        blocks = [int(v) for v in os.environ["DILATION_BLOCKS"].split(",")]
    assert all(b % 2 == 0 and b > 0 for b in blocks) and sum(blocks) == h
    first = int(os.environ.get("DILATION_FIRST", "6"))
    # tiny ungated prefix (chunk0 on q0, chunk1 on q2 in parallel) lands
    # before the first fold; chunk2 kept small so it beats blk2's need even
    # at the ~170GB/s effective rate of these strided row-loads
    loads = [first, 24, 40] + _block_sizes(h - first - 64, 3)
    if os.environ.get("DILATION_LOADS"):
        loads = [int(v) for v in os.environ["DILATION_LOADS"].split(",")]
    assert sum(loads) == h

    with TileContext(nc) as tc:
        with (
            tc.tile_pool(name="const", bufs=1) as cpool,
            tc.tile_pool(name="xp", bufs=1) as xpool,
            tc.tile_pool(name="accp", bufs=2) as apool,
            tc.tile_pool(name="tmpp", bufs=1) as tpool,
        ):
            se_t = cpool.tile([P, KH * KW], f32, name="se_t")
            se_dma = nc.sync.dma_start(out=se_t[:], in_=se_d[:])
            # own queue: don't chain the first x chunk behind the se load
            _FORCED_HW_QUEUE[se_dma.ins.name] = 5

            # One persistent padded-x tile: xt row t = padded-input row t.
            xt = xpool.tile([P, h + 2, W + 2], dt, name="xt")
            nc.vector.memset(xt[:, :, 0:1], CVAL)
            nc.vector.memset(xt[:, :, W + 1 : W + 2], CVAL)
            nc.vector.memset(xt[:, 0:1, :], CVAL)
            nc.vector.memset(xt[:, h + 1 : h + 2, :], CVAL)

            # Chunked loads, serially chained on HWDGE queue 0.
            load_dmas = []
            load_top = []  # last loaded input row (exclusive) per chunk
            y0 = 0
            for rows in loads:
                ld = nc.sync.dma_start(
                    out=xt[:, y0 + 1 : y0 + rows + 1, 1 : W + 1],
                    in_=x_d[:, y0 : y0 + rows, :],
                )
                # chunk1 rides its own queue, parallel with chunk0: both
                # land before the first fold (the ungated prefix stays tiny)
                _FORCED_HW_QUEUE[ld.ins.name] = 2 if len(load_dmas) == 1 else 0
                load_dmas.append(ld)
                y0 += rows
                load_top.append(y0)
            # Trip-and-hold avoidance (measured, mb3/mb7/mb8): an in-place DVE
            # op overlapping a SATURATED load stream latches the whole rest of
            # the run into a ~1.2x-slower mode (TT 8692->10429ns for the
            # remainder, even after DMA drains). Spread loads (~40% duty) or
            # out-of-place folds both avoid the trip. Gate chunk c behind an
            # early block's compute so the load stream never saturates while
            # folds run: release chunk c at block 2c-3 (consumed at block
            # 2c-1, ~2 blocks of lead).
            load_release = {}
            if not os.environ.get("DILATION_NOLDGATE"):
                lead = int(os.environ.get("DILATION_LDLEAD", "2"))
                bt = 0
                first_need = {}
                for bi, br in enumerate(blocks):
                    nt = min(bt + br + 1, h)
                    ci = next(i for i, top in enumerate(load_top) if top >= nt)
                    first_need.setdefault(ci, bi)
                    bt += br
                # chunks 0-1 stay ungated but are sized to land BEFORE the
                # first in-place fold; every later chunk is gated (release at
                # blk0's f01 at the earliest) so the load stream never
                # saturates while folds run. The trip is probabilistic: one
                # ungated-56-row config tripped on its first run while 15
                # runs of another didn't — keep the overlap at ~zero.
                for c in range(2, len(load_dmas)):
                    rb = max(first_need.get(c, 0) - lead, 0)
                    load_release.setdefault(rb, []).append(c)

            # scratch tiles for gates (tiny 1-element targets)
            dve_scr = cpool.tile([P, 3 * len(blocks)], dt, name="dve_scr")
            act_scr = cpool.tile([P, 4], dt, name="act_scr")
            act_src = cpool.tile([P, 1], dt, name="act_src")
            nc.vector.memset(act_src[:], 0.0)

            # ping-pong tmp tiles for the ACT adds
            maxrows = max(blocks)
            tmps = [
                tpool.tile([P, maxrows, W], dt, name=f"tmp{i}") for i in range(nslots)
            ]
            tmp_reader = [None] * nslots  # [0] = last fold of prev block

            out_dmas = []
            y0 = 0
            tmp_idx = 0
            for blk, rows in enumerate(blocks):
                # deepest load chunk this block needs (bottom halo row is
                # input row y0+rows; the queue-0 chain covers earlier chunks)
                need_top = min(y0 + rows + 1, h)
                ldi = next(i for i, top in enumerate(load_top) if top >= need_top)

                acc = apool.tile([P, rows, W], dt, name="acc")
                # DVE-side gates: gw absorbs the store whose acc slot this
                # block reuses, gx the input-chunk wait.
                if not os.environ.get("DILATION_NOGATES"):
                    gx = nc.vector.memset(dve_scr[:, 3 * blk : 3 * blk + 1], 0.0)
                    add_dep_helper(gx.ins, load_dmas[ldi].ins, reason="input chunk")
                # Hoist the first tmp_d tap's TS add (no acc dependency)
                # ahead of the store-WAR gates: hides ~1.6us of the acc-WAR
                # wait on blocks still inside the early-phase cascade (trace:
                # 7.3us DVE stall at blk 3 on a store-half sem).
                tmp_d = tpool.tile([P, rows, W], dt, name="tmp_d")
                (h_i, h_j) = _DVE_TAPS[1]
                nc.vector.tensor_scalar(
                    tmp_d[:],
                    xt[:, y0 + h_i : y0 + h_i + rows, h_j : h_j + W],
                    se_t[:, 3 * h_i + h_j : 3 * h_i + h_j + 1],
                    None,
                    add,
                )
                if blk >= 2 and not os.environ.get("DILATION_NOWARGATES"):
                    gw = nc.vector.memset(dve_scr[:, 3 * blk + 1 : 3 * blk + 2], 0.0)
                    add_dep_helper(gw.ins, out_dmas[blk - 2][0].ins, reason="acc WAR")
                    gw2 = nc.vector.memset(dve_scr[:, 3 * blk + 2 : 3 * blk + 3], 0.0)
                    add_dep_helper(gw2.ins, out_dmas[blk - 2][1].ins, reason="acc WAR2")
                # ACT-side gate for the input chunk (294ns each on the
                # bottleneck engine; NOACTGATES drops them in favor of auto
                # region deps + wait-split drains)
                if not os.environ.get("DILATION_NOACTGATES"):
                    ga = nc.scalar.activation(
                        act_scr[:, 0:1], act_src[:, 0:1], ident, bias=se_t[:, 0:1]
                    )
                    add_dep_helper(ga.ins, load_dmas[ldi].ins, reason="input chunk/ACT")

                def act_add(tap, gate_tt, split=0):
                    """Emit one ACT add into the next tmp slot; gate_tt (if
                    set) is a fold whose completion frees the slot (and, by
                    the cumulative DVE semaphore, every earlier fold). With
                    split>0 and an even-offset tap, the bottom `split` rows
                    are computed by an otherwise-idle DVE 4x tensor_scalar
                    instead (rebalances the ACT-bound pipeline); the fold's
                    DVE-chain wait covers that part for free."""
                    nonlocal tmp_idx
                    t_i, t_j = tap
                    ti = tmp_idx % nslots
                    tmp_idx += 1
                    sidx = 3 * t_i + t_j
                    if gate_tt is not None and not os.environ.get(
                        "DILATION_NOACTGATES"
                    ):
                        gt = nc.scalar.activation(
                            act_scr[:, 1:2], act_src[:, 0:1], ident,
                            bias=se_t[:, 0:1],
                        )
                        add_dep_helper(gt.ins, gate_tt.ins, reason="tmp WAR gate")
                    arows = rows - split
                    a = nc.scalar.activation(
                        tmps[ti][:, 0:arows, :],
                        xt[:, y0 + t_i : y0 + t_i + arows, t_j : t_j + W],
                        ident,
                        bias=se_t[:, sidx : sidx + 1],
                    )
                    if split:
                        nc.vector.tensor_scalar(
                            tmps[ti][:, arows:rows, :],
                            xt[:, y0 + t_i + arows : y0 + t_i + rows, t_j : t_j + W],
                            se_t[:, sidx : sidx + 1],
                            None,
                            add,
                        )
                    return ti, a

                def act_fold_pair(p0, p1):
                    """Fold two finished ACT tmps; one gate on the later add
                    covers both (same-engine retirement is in-order)."""
                    if not os.environ.get("DILATION_NOGATES"):
                        gm = nc.vector.memset(dve_scr[:, 3 * blk : 3 * blk + 1], 0.0)
                        add_dep_helper(gm.ins, p1[1].ins, reason="ACT pair ready")
                    nc.vector.tensor_tensor(
                        acc[:], acc[:], tmps[p0[0]][:, 0:rows, :], vmax
                    )
                    return nc.vector.tensor_tensor(
                        acc[:], acc[:], tmps[p1[0]][:, 0:rows, :], vmax
                    )

                # DVE-only taps: aligned TS init + 2x (TS add -> tmp_d, TT max)
                # (first tmp_d TS was hoisted above the store-WAR gates)
                (i0, j0) = _DVE_TAPS[0]
                nc.vector.tensor_scalar(
                    acc[:],
                    xt[:, y0 + i0 : y0 + i0 + rows, j0 : j0 + W],
                    se_t[:, 3 * i0 + j0 : 3 * i0 + j0 + 1],
                    None,
                    add,
                )
                nc.vector.tensor_tensor(acc[:], acc[:], tmp_d[:], vmax)
                (t_i, t_j) = _DVE_TAPS[2]
                sidx = 3 * t_i + t_j
                nc.vector.tensor_scalar(
                    tmp_d[:],
                    xt[:, y0 + t_i : y0 + t_i + rows, t_j : t_j + W],
                    se_t[:, sidx : sidx + 1],
                    None,
                    add,
                )
                nc.vector.tensor_tensor(acc[:], acc[:], tmp_d[:], vmax)
                # paired folds with slot refill: adds a0(s0) a1(s1) fold01,
                # a2(s2) a3(s0) fold23, a4(s1) a5(s2) fold45. Slot-reuse gates
                # wait on the later fold of the freeing pair (cumulative).
                sp01 = 2 * (int(os.environ.get("DILATION_SP01", "0")) // 2)
                sp01 = max(0, min(sp01, rows - 2))  # keep the ACT part >= 2 rows
                p0 = act_add(_ACT_TAPS[0], tmp_reader[0], split=sp01)
                p1 = act_add(_ACT_TAPS[1], None, split=sp01)
                f01 = act_fold_pair(p0, p1)
                for c in load_release.get(blk, []):
                    add_dep_helper(
                        load_dmas[c].ins, f01.ins, reason="load throttle"
                    )
                p2 = act_add(_ACT_TAPS[2], None)
                # sp=4 (spdiv=10) measured best: DVE is the binding engine, so
                # keep the ACT->DVE row-split small (395us vs 397 at sp=6,
                # 404 at sp=8, 408 at sp=0).
                spdiv = int(os.environ.get("DILATION_SPDIV", "10"))
                sp = 2 * (rows // spdiv)  # even bottom slice for the DVE part
                if blk == len(blocks) - 1:
                    # last block: DVE idles at the drain; absorb more ACT rows
                    tsp = int(os.environ.get("DILATION_TAILSP", "6"))
                    sp = min(rows, 2 * (tsp // 2))
                if blk == 0 and os.environ.get("DILATION_SP0"):
                    # Tested: DVE-heavy first block (sp0=16) = 399.6us, +8us
                    # WORSE — DVE is the slower engine, so absorbing ACT rows
                    # in block 0 delays its own folds 1:1; the early ACT-wait
                    # gaps are already mostly pipeline-hidden.
                    sp = int(os.environ["DILATION_SP0"])
                p3 = act_add(_ACT_TAPS[3], f01, split=sp)  # s0 freed by f01
                f23 = act_fold_pair(p2, p3)
                p4 = act_add(_ACT_TAPS[4], None)  # s1 freed by f01 (observed)
                p5 = act_add(_ACT_TAPS[5], f23, split=sp)  # s2 freed by f23
                f45 = act_fold_pair(p4, p5)
                # s0's last reader is f23's 2nd TT (f45 only reads s1/s2), so
                # gating next block's a0 on f23 lets ACT start a block earlier
                # relative to DVE; a1/a2 WAR vs f45 ride their auto region
                # deps. (Trace showed ACT-lag-induced DVE fold stalls in the
                # first ~85us.)
                tmp_reader[0] = f45 if os.environ.get("DILATION_F45") else f23

                # two half-height stores on separate queues: acc's WAR
                # frees ~2.7us sooner, damping the early-phase store-WAR
                # cascade; all stores HWDGE (queues recycle every ~3.5 blocks)
                if blk == len(blocks) - 1 and rows % 4 == 0:
                    # tail: 4 parallel quarter-stores shorten the final drain
                    q4 = rows // 4
                    tail_stores = []
                    for qi in range(4):
                        od = nc.sync.dma_start(
                            out=out_d[:, y0 + qi * q4 : y0 + (qi + 1) * q4, :],
                            in_=acc[:, qi * q4 : (qi + 1) * q4, :],
                        )
                        _FORCED_HW_QUEUE[od.ins.name] = 1 + (2 * blk + qi) % _STQN
                        tail_stores.append(od)
                    out_dmas.append((tail_stores[0], tail_stores[-1]))
                else:
                    h1 = rows // 2
                    od1 = nc.sync.dma_start(
                        out=out_d[:, y0 : y0 + h1, :], in_=acc[:, 0:h1, :]
                    )
                    _FORCED_HW_QUEUE[od1.ins.name] = 1 + (2 * blk) % _STQN
                    od2 = nc.sync.dma_start(
                        out=out_d[:, y0 + h1 : y0 + rows, :], in_=acc[:, h1:rows, :]
                    )
                    _FORCED_HW_QUEUE[od2.ins.name] = 1 + (2 * blk + 1) % _STQN
                    out_dmas.append((od1, od2))
                y0 += rows

    if split_waits:
        _split_excess_waits(nc, mybir)
    return nc


def _get_nc():
    key = (_DTYPE,)
    if key not in _nc_cache:
        _nc_cache[key] = _build(
            _DTYPE, nblocks=int(os.environ.get("DILATION_NBLOCKS", "11"))
        )
    return _nc_cache[key]


def kernel(x: np.ndarray, se: np.ndarray) -> np.ndarray:
    global LAST_RESULTS
    from concourse.bass_utils import run_bass_kernel_spmd

    np_dt = np.float16 if _DTYPE == "f16" else np.float32
    x = np.asarray(x)
    se = np.asarray(se)
    xs = np.ascontiguousarray(x).reshape(NCORES, P, H, W).astype(np_dt)
    sep = np.ascontiguousarray(
        np.tile(np.asarray(se, np.float32).reshape(C, KH * KW), (P // C, 1))
    )

    nc = _get_nc()
    in_maps = [{"x": xs[k], "sep": sep} for k in range(NCORES)]
    trace = bool(os.environ.get("DILATION_TRACE"))
    kwargs = {}
    if trace:
        kwargs["trace"] = True
        tmpdir = os.environ.get("DILATION_TRACE_DIR")
        if tmpdir:
            kwargs["tmpdir"] = tmpdir
    res = run_bass_kernel_spmd(nc, in_maps, list(range(NCORES)), **kwargs)
    LAST_RESULTS = res
    out = np.stack([res.results[k]["out"] for k in range(NCORES)])
    return out.reshape(B, C, H, W).astype(np.float32)

